# revision 27
# baseline (speedup 1.0000x reference)
"""Trainium2 Bass kernel for nn_BlockV3 (dense transformer block).

Sharding: 8 cores = 2 (batch) x 4 (query-quarter). Each core holds the full
batch element for K/V and computes attention + MLP for its own 512 query
rows. Host-side prep reorders tokens per core (own 512 first) so the device
program is identical across cores (SPMD).

v5 design (on top of v3):
  - attention projections (Q/K/V/out) run fp8e4 DoubleRow (weights x64 to
    dodge fp8 subnormals, descale fused into the bias op). The MLP runs
    bf16.
  - score exp batched over both heads of a pair: one ACT exp per key tile
    over a 2-bank [128,1024] PSUM tile.
  - ACT table discipline: ONE Ln + ONE Exp for all four LN1 quarters
    (before the exp stream), 1/den on DVE via reciprocal_approx_fast,
    LN2 Ln+Exp + Gelu at the tail: 5 ACT_TABLE_LOADs total (v3 had 13)
    and a fully uninterrupted 96-exp stream.
  - LN1 stat matmuls use a one-hot [P,64] lhsT so the unused psum rows
    are written to exact zero: the junk-lane clamps (3 extra DVE row ops
    on the critical LN chain) disappear.
  - x loaded in 6 big 524KB chunk DMAs, consts packed into two DMAs,
    vmask DMAs replaced by one DVE op per v tile; stats run chunk-major,
    pacing with the x DMA stream.
  - PE warmup dummy matmuls bridge the x-DMA wait AND the LN-chain stall
    so HAM stays at K=8/8 into the score stream.
  - SBUF->SBUF elementwise work (LN apply, x2/xsq squares, u2) is split
    with the otherwise-idle GpSimd engine to unclog DVE, which gates the
    attention ramp.
  - den reciprocal + y-division (finish_b) run INSIDE the attention loop
    (pair hp-2 at iteration hp), so the out-projection can fire almost
    immediately after the last attV matmul.
  - out-projection j-major over sp2/mm psum rings freed by attention;
    per-ot epilogues pipeline behind the remaining chains; MLP final
    window finishes per-ot so output DMAs start early.
"""

import sys
import numpy as np

sys.path.insert(0, "/opt/trn_rl_repo")

B = 2
T = 2048
C = 768
H = 12
Dh = 64
F = 3072
P = 128
NCH = C // P          # 6 feature chunks
NP = NCH // 2         # 3 chunk pairs (DoubleRow)
NFT = F // P          # 24 mlp chunks
NKT = T // P          # 16 key tiles
TQ = 512              # own query rows per core
NQ4 = T // TQ         # 4 t-quarters
N_CORES = 8
EPS = 1e-5
WSC = 64.0            # host-side fp8 weight scale (attention mats only)
WDESC = 1.0 / WSC
NWARM = 14            # PE warmup dummy matmuls at t=0

_CACHE = {}

# constpack column layout (f32, [P, 64])
CP_MB = 0             # 16 cols: mask/64 per key tile
CP_BQ = 16            # 6
CP_BK = 22            # 6
CP_B1 = 34            # 24
CP_B2 = 58            # 6


def _build_nc():
    import concourse.bass as bass
    from concourse import bacc, mybir
    import concourse.tile as tile

    f32 = mybir.dt.float32

    nc = bacc.Bacc()
    eps_t = nc.alloc_sbuf_tensor("const-eps", [128, 1], f32)
    nc.gpsimd.memset(eps_t.ap(), EPS)
    nc.const_aps.aps[(f32, EPS)] = eps_t.ap()

    f8 = mybir.dt.float8e4
    bf16 = mybir.dt.bfloat16
    d = {}
    d["xT"] = nc.declare_dram_parameter("xT", [C, T], bf16, isOutput=False)
    d["xTown"] = nc.declare_dram_parameter("xTown", [C, TQ], f32, isOutput=False)
    d["cpk"] = nc.declare_dram_parameter("cpk", [P, 64], f32, isOutput=False)
    d["spk"] = nc.declare_dram_parameter("spk", [65, 5, P], bf16, isOutput=False)
    d["wqB"] = nc.declare_dram_parameter("wqB", [NCH, P, NCH, P], f8, isOutput=False)
    d["wkB"] = nc.declare_dram_parameter("wkB", [NCH, P, NCH, P], f8, isOutput=False)
    d["wvP"] = nc.declare_dram_parameter("wvP", [NP, P, 2, C], f8, isOutput=False)
    d["wpB"] = nc.declare_dram_parameter("wpB", [NCH, P, NCH, P], f8, isOutput=False)
    d["w1B"] = nc.declare_dram_parameter("w1B", [NFT, P, NCH, P], bf16, isOutput=False)
    d["w2B"] = nc.declare_dram_parameter("w2B", [NCH, P, NFT, P], bf16, isOutput=False)
    d["outT"] = nc.declare_dram_parameter("outT", [C, TQ], f32, isOutput=True)

    with tile.TileContext(nc) as tc:
        _emit(tc, nc, mybir, bass, tile, d)
    nc.finalize()
    return nc


def _emit(tc, nc, mybir, bass, tile, g):
    from contextlib import ExitStack

    f32 = mybir.dt.float32
    bf16 = mybir.dt.bfloat16
    f8 = mybir.dt.float8e4
    AF = mybir.ActivationFunctionType
    OP = mybir.AluOpType
    DR = mybir.MatmulPerfMode.DoubleRow
    ts = bass.ts

    xT, xTown = g["xT"], g["xTown"]
    cpkD, spkD = g["cpk"], g["spk"]
    wqB, wkB, wvP, wpB, w1B, w2B = (g["wqB"], g["wkB"], g["wvP"], g["wpB"],
                                    g["w1B"], g["w2B"])
    outT = g["outT"]

    ctx = ExitStack()
    with ctx:
        sb = ctx.enter_context(tc.tile_pool(name="sb", bufs=1))

        def st(shape, dtype, tag, bufs, name):
            return sb.tile(shape, dtype, tag=tag, bufs=bufs, name=name)

        # ---- input x entirely over the GpSimd SWDGE path: consecutive
        # SWDGE transfers overlap in the SDMA engines (~250GB/s measured)
        # while the sync HWDGE ring serializes at ~100GB/s. The sync ring
        # keeps the small weights/consts. ----
        xt = [st([P, T], bf16, "xt", NCH, f"xt{c}") for c in range(NCH)]
        for c in range(NCH):
            nc.gpsimd.dma_start(xt[c], xT[c * P:(c + 1) * P, :])
        cpk = st([P, 64], f32, "cpk", 1, "cpk")
        nc.sync.dma_start(cpk, cpkD[:, :])
        mb = cpk[:, CP_MB:CP_MB + NKT]
        bq_s = cpk[:, CP_BQ:CP_BQ + NCH]
        bk_s = cpk[:, CP_BK:CP_BK + NCH]
        b1_s = cpk[:, CP_B1:CP_B1 + NFT]
        b2_s = cpk[:, CP_B2:CP_B2 + NCH]
        spk = st([65, 5, P], bf16, "spk", 1, "spk")
        nc.sync.dma_start(spk, spkD[:, :, :])
        sel_s = spk[0:2, 0, :]
        selq_s = [spk[:, 1 + i, :] for i in range(4)]

        # ---- small on-device consts (junk first: warmup reads it) ----
        junk = st([P, TQ], bf16, "ab", 8, "junk")
        nc.vector.memset(junk, 0.0)
        # 1-element Exp pulls the exp table load to t~7us (ACT is idle),
        # removing the 1.3us ACT_TABLE_LOAD from the LN chain's serial path
        nc.scalar.activation(junk[0:1, 0:1], junk[0:1, 0:1],
                             mybir.ActivationFunctionType.Exp, bias=0.0,
                             scale=1.0)
        ones_b = st([P, 1], bf16, "ones_b", 1, "ones_b")
        nc.vector.memset(ones_b, 1.0)
        ones_rf = st([1, P], bf16, "ones_rf", 1, "ones_rf")
        nc.vector.memset(ones_rf, 1.0)
        neg_rf = st([1, P], bf16, "neg_rf", 1, "neg_rf")
        nc.vector.memset(neg_rf, -1.0)
        ones_h = st([P, H, 1], bf16, "ones_h", 1, "ones_h")
        nc.vector.memset(ones_h, 1.0)
        # one-hot stats lhsT: col 0 ones, cols 1-63 zero -> a [64, TQ]
        # matmul output whose rows 1-63 are EXACT zeros (junk-lane safety
        # for the LN chain without clamps)
        oh = st([P, 64], bf16, "oh", 1, "oh")
        nc.vector.memset(oh, 0.0)
        nc.vector.memset(oh[:, 0:1], 1.0)

        x2t = []
        u2 = []

        with tc.tile_pool(name="psum", bufs=2, space="PSUM") as psum:

            def pmm(name):
                return psum.tile([P, TQ], f32, tag="mm", bufs=2, name=name)

            def pya(name):
                return psum.tile([P, TQ], f32, tag="ya", bufs=2, name=name)

            def psp(name):
                return psum.tile([P, TQ], f32, tag="sp2", bufs=2, name=name)

            _dm = [0]

            def dummies(n, tag="sp2"):
                # PE keep-warm filler: harmless matmuls on the junk tile
                # into a fresh ring psum tile. They absorb PE idle at
                # known stalls so HAM stays at K=8/8.
                _dm[0] += 1
                dm = psum.tile([P, TQ], f32, tag=tag, bufs=2,
                               name=f"dm{_dm[0]}")
                for i in range(n):
                    nc.tensor.matmul(dm, junk[:, 0:P], junk, start=True,
                                     stop=True, skip_group_check=True)

            # ---- PE warmup during the x DMA wait ----
            dummies(NWARM)

            def ln_rows(s12_aps, nm, rows=1):
                """n pairs of [rows,TQ] f32 psum sums -> (a, b) [rows,n,TQ]
                bf16 tiles. a = rsqrt(var+eps) = exp(-0.5*ln(var+eps)),
                b = mu*a (negated via neg one-hot at broadcast). All pairs
                share ONE Ln and ONE Exp so each ACT table loads once."""
                n = len(s12_aps)
                mu = st([rows, n, TQ], f32, "row", 3, nm + "mu")
                vpe = st([rows, n, TQ], f32, "row", 3, nm + "v")
                musq = st([rows, n, TQ], f32, "row", 3, nm + "m2")
                for i, (s1_ap, s2_ap) in enumerate(s12_aps):
                    nc.vector.tensor_scalar_mul(mu[:, i, :], s1_ap, 1.0 / C)
                    nc.vector.tensor_scalar(vpe[:, i, :], s2_ap, 1.0 / C,
                                            EPS, OP.mult, OP.add)
                    nc.vector.tensor_tensor(musq[:, i, :], mu[:, i, :],
                                            mu[:, i, :], OP.mult)
                    nc.vector.tensor_tensor(vpe[:, i, :], vpe[:, i, :],
                                            musq[:, i, :], OP.subtract)
                # junk rows hold var=EPS, mu=0 (one-hot stats zeros) so no
                # clamps are needed; Ln/Exp in place, b reuses musq.
                nc.scalar.activation(vpe, vpe, AF.Ln, bias=0.0, scale=1.0)
                nc.scalar.activation(vpe, vpe, AF.Exp, bias=0.0, scale=-0.5)
                ab = st([rows, n, TQ], bf16, "rowb", 2, nm + "ab")
                nc.vector.tensor_copy(ab, vpe)
                nc.vector.tensor_tensor(musq, mu, vpe, OP.mult)
                bb = st([rows, n, TQ], bf16, "rowb", 2, nm + "bb")
                nc.vector.tensor_copy(bb, musq)
                return ab, bb

            def bcast128(row, negate, name):
                """[1,TQ] bf16 row -> [128,TQ] bf16 tile via K=1 matmul."""
                pp = psp(name + "p")
                nc.tensor.matmul(pp, neg_rf if negate else ones_rf, row,
                                 start=True, stop=True)
                out = st([P, TQ], bf16, "ab", 8, name)
                nc.vector.tensor_copy(out, pp)
                return out

            # ======= Phase A: LN1 stats chunk-major + u1 (fp8 pairs) =========
            u1p = [st([P, 2, T], f8, "u1p", NP, f"u1p{j}") for j in range(NP)]

            kt = []
            wks = []
            for ot in range(NCH):
                kt.append(st([P, T], bf16, "kt", NCH, f"kt{ot}"))
                wks.append(None)

            def emit_k_weight(ot):
                w = st([P, NCH, P], f8, "w15", 8, f"wk{ot}")
                nc.sync.dma_start(w, wkB[ot])
                wks[ot] = w

            emit_k_weight(0)
            emit_k_weight(1)

            # Q weights (needed right after the LN chain)
            wqs = []
            for ot in range(NCH):
                wq = st([P, NCH, P], f8, "w15", 8, f"wq{ot}")
                nc.sync.dma_start(wq, wqB[ot])
                wqs.append(wq)

            # V weights
            wv = []
            for j in range(NP):
                w = st([P, 2, C], f8, "wv", NP, f"wv{j}")
                nc.sync.dma_start(w, wvP[j])
                wv.append(w)

            # stats: s12a (mm ring) for quarters 0/1, s12b (ya ring) for 2/3
            s12a = (pmm("s1p01"), pmm("s2p01"))
            s12b = (pya("s1p23"), pya("s2p23"))

            for c in range(NCH):
                for q in range(NQ4):
                    s1q, s2q = s12a if q < 2 else s12b
                    r = (q % 2) * 64
                    xq = xt[c][:, ts(q, TQ)]
                    xsq = st([P, TQ], bf16, "xsq", 3, f"xsq{q}_{c}")
                    nc.vector.tensor_tensor(xsq, xq, xq, OP.mult)
                    nc.tensor.matmul(s1q[r:r + 64, :], oh, xq,
                                     start=(c == 0), stop=(c == NCH - 1),
                                     skip_group_check=True)
                    nc.tensor.matmul(s2q[r:r + 64, :], oh, xsq,
                                     start=(c == 0), stop=(c == NCH - 1),
                                     skip_group_check=True)

            # cover the LN-chain DVE/ACT latency on the PE side
            dummies(30)

            def bcast128q(row_pair, r, negate, name):
                # broadcast row r of a [65,TQ] pair via a one-hot K=65 matmul
                pp = psp(name + "p")
                nc.tensor.matmul(pp, selq_s[2 * negate + r], row_pair,
                                 start=True, stop=True)
                out = st([P, TQ], bf16, "ab", 8, name)
                nc.vector.tensor_copy(out, pp)
                return out

            # chain-a (quarters 0/1) first so stream_q(0) launches ASAP;
            # chain-b lands in hp0's ACT slack (2 extra table loads, but
            # the exp stream is PE-bound there anyway).
            ab4s = [None] * NQ4

            def emit_chain(qp, s12):
                a_r, b_r = ln_rows(
                    [(s12[0][0:65, :], s12[1][0:65, :])], f"r{qp}", rows=65)
                for r in range(2):
                    q = 2 * qp + r
                    a4 = bcast128q(a_r[:, 0, :], r, 0, f"a4_{q}")
                    b4 = bcast128q(b_r[:, 0, :], r, 1, f"b4_{q}")
                    ab4s[q] = (a4, b4)

            emit_chain(0, s12a)
            dummies(10)

            def phase_a_apply(q):
                # u1 = x*a + b, chunk-parity split across DVE and the
                # otherwise idle GpSimd (both SBUF->SBUF)
                a4, b4 = ab4s[q]
                for c in range(NCH):
                    tmpu = st([P, TQ], bf16, "tmpu", 2, f"tmpu{q}_{c}")
                    nc.vector.tensor_tensor(tmpu, xt[c][:, ts(q, TQ)], a4,
                                            OP.mult)
                    nc.vector.tensor_tensor(u1p[c // 2][:, c % 2, ts(q, TQ)],
                                            tmpu, b4, OP.add)

            # ======= projections (fp8 DoubleRow) =============================
            qt = []

            def emit_q_proj():
                for ot in range(NCH):
                    qp = pmm(f"qp{ot}")
                    for j in range(NP):
                        nc.tensor.matmul(qp, wqs[ot][:, 2 * j:2 * j + 2, :],
                                         u1p[j][:, :, 0:TQ],
                                         start=(j == 0), stop=(j == NP - 1),
                                         perf_mode=DR)
                    qs = st([P, TQ], bf16, "qu", NCH, f"qt{ot}")
                    nc.vector.tensor_scalar(qs, qp, WDESC, bq_s[:, ot:ot + 1],
                                            OP.mult, OP.add)
                    qt.append(qs)

            def emit_k_quarter(ot, gq):
                kp = pmm(f"kp{ot}_{gq}")
                for j in range(NP):
                    nc.tensor.matmul(kp, wks[ot][:, 2 * j:2 * j + 2, :],
                                     u1p[j][:, :, ts(gq, TQ)],
                                     start=(j == 0), stop=(j == NP - 1),
                                     perf_mode=DR)
                nc.vector.tensor_scalar(kt[ot][:, ts(gq, TQ)], kp, WDESC,
                                        bk_s[:, ot:ot + 1], OP.mult, OP.add)

            # V: token-major v [T, C] with the 0/1 mask folded in: masked rows
            # zeroed, per-head 65th column = mask/64, so att@v yields the
            # masked numerator and denominator with unmasked exp.
            vt = [None] * (NKT // 2)

            def emit_v_tile(tk):
                va = pmm(f"vpa{tk}")
                vb = pmm(f"vpb{tk}")[:, 0:256]
                for j in range(NP):
                    lhs = u1p[j][:, :, ts(tk, P)]
                    nc.tensor.matmul(va, lhs, wv[j][:, :, 0:512],
                                     start=(j == 0), stop=(j == NP - 1),
                                     perf_mode=DR)
                    nc.tensor.matmul(vb, lhs, wv[j][:, :, 512:768],
                                     start=(j == 0), stop=(j == NP - 1),
                                     perf_mode=DR)
                if tk % 2 == 0:
                    vt[tk // 2] = st([P, 2, H, 68], f8, "vp", NKT // 2,
                                     f"v{tk // 2}")
                v = vt[tk // 2][:, tk % 2, :, :]
                va3 = va.rearrange("p (h d) -> p h d", d=64)
                vb3 = vb.rearrange("p (h d) -> p h d", d=64)
                # mb holds mask/64 so this applies mask AND the fp8 weight
                # descale; the /64 on the den column cancels in recip_rows.
                mcol = mb[:, tk:tk + 1]
                nc.vector.tensor_scalar_mul(v[:, 0:8, 0:64], va3, mcol)
                nc.vector.tensor_scalar_mul(v[:, 8:12, 0:64], vb3, mcol)
                # den column: mask/64 replicated over the 12 heads
                nc.vector.tensor_scalar_mul(v[:, :, 64:65], ones_h, mcol)

            # ystack: fp8 y (divided by den), chunk pairs for the DoubleRow
            # out-projection. Chunk hp at [:, hp%2, :] of tile hp//2.
            ystack = [st([P, 2, TQ], f8, "wv", NP, f"ystack{j}")
                      for j in range(NP)]
            ybf = [None] * NCH
            # denominators for all 6 pairs: [2, NCH*TQ] rows
            den_all = st([2, NCH * TQ], bf16, "den", 1, "den_all")

            def scores_exp(hp, ets_gen, tk):
                sp2 = psum.tile([P, 2 * TQ], f32, tag="sp2", bufs=2,
                                name=f"sp2_{hp}_{tk}")
                for h2 in range(2):
                    rows = slice(64 * h2, 64 * h2 + 64)
                    nc.tensor.matmul(sp2[:, ts(h2, TQ)],
                                     kt[hp][rows, ts(tk, P)],
                                     qt[hp][rows, :], start=True, stop=True)
                if tk % 2 == 0:
                    ets_gen[tk // 2] = st([P, 2, 2, TQ], f8, "et", 16,
                                          f"et{hp}_{tk // 2}")
                nc.scalar.activation(ets_gen[tk // 2][:, :, tk % 2, :], sp2,
                                     AF.Exp, bias=0.0, scale=0.125)

            def finish_a(hp, yp):
                """Copy y (undivided) + den out of PSUM; division deferred
                to recip_rows/finish_b."""
                ybf[hp] = st([P, TQ], bf16, "ybf", NCH, f"ybf{hp}")
                for h2 in range(2):
                    yc = st([65, TQ], bf16, "yc", 2, f"yc{2 * hp + h2}")
                    nc.vector.tensor_copy(yc, yp[h2])
                    nc.sync.dma_start(den_all[h2:h2 + 1, ts(hp, TQ)],
                                      yc[64:65, :])
                    nc.sync.dma_start(ybf[hp][64 * h2:64 * h2 + 64, :],
                                      yc[0:64, :])

            def recip_rows(hp):
                # 1/den for pair hp on DVE via the 51-ULP fast reciprocal
                # (no ACT table, no ACT queue time). den_all holds den/64,
                # so scale by 64 going to f32 and 1/x lands on 1/den_true.
                sl = slice(hp * TQ, (hp + 1) * TQ)
                cv = st([2, TQ], f32, "lden", 2, f"ldc{hp}")
                nc.vector.tensor_scalar_mul(cv, den_all[:, sl], WSC)
                rc = st([2, TQ], f32, "lden", 2, f"ldr{hp}")
                nc.vector.reciprocal_approx_fast(rc, cv)
                nc.vector.tensor_copy(den_all[:, sl], rc)

            def finish_b(hp, ring=pmm):
                rp = ring(f"rp{hp}")
                nc.tensor.matmul(rp, sel_s, den_all[:, ts(hp, TQ)],
                                 start=True, stop=True)
                rb = st([P, TQ], bf16, "rb", 2, f"rb{hp}")
                nc.vector.tensor_copy(rb, rp)
                nc.vector.tensor_tensor(ystack[hp // 2][:, hp % 2, :],
                                        ybf[hp], rb, OP.mult)

            # ================= fused LN1 + QKV + attention ===================
            ets_prev = None
            ets_gen = [None] * (NKT // 2)

            def stream_q(q):
                phase_a_apply(q)
                if q == 0:
                    emit_q_proj()
                emit_k_quarter(0, q)
                for tk in range(4 * q, 4 * q + 4):
                    scores_exp(0, ets_gen, tk)
                    if q > 1:
                        # v tiles for tks 0-7 are deferred into the hp=1
                        # loop (the gp order i+4 reads them last); this
                        # slims the PE-heaviest stretch of the ramp
                        emit_v_tile(tk)
                emit_k_quarter(1, q)

            stream_q(0)
            emit_chain(1, s12b)
            stream_q(1)
            stream_q(2)
            stream_q(3)
            ets_prev = ets_gen

            # residuals for the out-proj epilogue (bp folded in on host)
            xos = []
            for ot in range(NCH):
                xo = st([P, TQ], f32, "xtown", 3, f"xo{ot}")
                nc.sync.dma_start(xo, xTown[ot * P:(ot + 1) * P, :])
                xos.append(xo)

            wps = [None] * NCH

            for hp in range(1, NCH):
                ets_gen = [None] * (NKT // 2)
                yas = [pya(f"ya{2 * (hp - 1) + h2}")[0:65, :]
                       for h2 in range(2)]
                if hp + 1 < NCH:
                    emit_k_weight(hp + 1)
                if hp == 3:
                    for ot in range(NCH):
                        wp = st([P, NCH, P], f8, "w15", 8, f"wp{ot}")
                        nc.sync.dma_start(wp, wpB[ot])
                        wps[ot] = wp
                for tk in range(NKT):
                    scores_exp(hp, ets_gen, tk)
                    if hp == 1 and tk in (1, 3, 5, 7):
                        emit_v_tile(tk - 1)
                        emit_v_tile(tk)
                    if tk % 2 == 1:
                        i = tk // 2
                        gp = (i + 4) % (NKT // 2)
                        for h2 in range(2):
                            nc.tensor.matmul(
                                yas[h2],
                                vt[gp][:, :, 2 * (hp - 1) + h2, 0:65],
                                ets_prev[gp][:, h2, :, :],
                                start=(i == 0), stop=(i == NKT // 2 - 1),
                                perf_mode=DR)
                    if hp + 1 < NCH and tk % 4 == 3:
                        emit_k_quarter(hp + 1, tk // 4)
                finish_a(hp - 1, yas)
                if hp >= 2:
                    # divide pair hp-2 (its den DMA landed an iteration
                    # ago) while attention still runs
                    recip_rows(hp - 2)
                    finish_b(hp - 2)
                ets_prev = ets_gen

            # tail attV for the last head pair; pair-4 division and the
            # j=0/1 out-projection chains run in its shadow
            yas = [pya(f"ya{2 * (NCH - 1) + h2}")[0:65, :] for h2 in range(2)]
            for i in range(NKT // 2):
                gp = (i + 4) % (NKT // 2)
                for h2 in range(2):
                    nc.tensor.matmul(
                        yas[h2], vt[gp][:, :, 2 * (NCH - 1) + h2, 0:65],
                        ets_prev[gp][:, h2, :, :],
                        start=(i == 0), stop=(i == NKT // 2 - 1),
                        perf_mode=DR)
            recip_rows(NCH - 2)
            xpA = psum.tile([P, 2 * TQ], f32, tag="sp2", bufs=2, name="xpA")
            xpB = psum.tile([P, 2 * TQ], f32, tag="sp2", bufs=2, name="xpB")
            xps = [pmm("xp0"), pmm("xp1"), xpA[:, 0:TQ], xpA[:, TQ:2 * TQ],
                   xpB[:, 0:TQ], xpB[:, TQ:2 * TQ]]
            for j in range(2):
                for ot in range(NCH):
                    nc.tensor.matmul(xps[ot], wps[ot][:, 2 * j:2 * j + 2, :],
                                     ystack[j],
                                     start=(j == 0), stop=False,
                                     perf_mode=DR)
            finish_a(NCH - 1, yas)
            # pairs 4/5: the mm ring is held by xp0/xp1 now, so their den
            # broadcasts ride the ya ring (freed by finish_a above)
            finish_b(NCH - 2, ring=pya)
            recip_rows(NCH - 1)
            finish_b(NCH - 1, ring=pya)
            dummies(8, tag="ya")

            # ============ out-projection tail + residual + LN2 ===============
            for ot in range(NCH):
                nc.tensor.matmul(xps[ot], wps[ot][:, 4:6, :],
                                 ystack[2],
                                 start=False, stop=True,
                                 perf_mode=DR)
            s1q = pmm("s1q_ln2")
            s2q = pmm("s2q_ln2")
            for ot in range(NCH):
                # x2 = xp/64 + (x + bp)   (bp folded into xTown on host)
                x2 = st([P, TQ], f32, "xt", NCH, f"x2t{ot}")
                nc.vector.scalar_tensor_tensor(x2, xps[ot], WDESC, xos[ot],
                                               OP.mult, OP.add)
                x2t.append(x2)
                x2b = st([P, TQ], bf16, "x2b", 4, f"x2b{ot}")
                nc.vector.tensor_copy(x2b, x2)
                xsq = st([P, TQ], bf16, "xsq2t", 4, f"xsq2_{ot}")
                nc.vector.tensor_tensor(xsq, x2b, x2b, OP.mult)
                nc.tensor.matmul(s1q[0:64, :], oh, x2b,
                                 start=(ot == 0), stop=(ot == NCH - 1),
                                 skip_group_check=True)
                nc.tensor.matmul(s2q[0:64, :], oh, xsq,
                                 start=(ot == 0), stop=(ot == NCH - 1),
                                 skip_group_check=True)
            dummies(14, tag="ya")
            a_r2, b_r2 = ln_rows([(s1q[0:1, :], s2q[0:1, :])], "ln2", rows=1)
            a2b = bcast128(a_r2[:, 0, :], False, "a2b")
            b2b = bcast128(b_r2[:, 0, :], True, "b2b")
            for c in range(NCH):
                u = st([P, TQ], bf16, "qu", NCH, f"u2_{c}")
                nc.vector.tensor_tensor(u, x2t[c], a2b, OP.mult)
                nc.vector.tensor_tensor(u, u, b2b, OP.add)
                u2.append(u)

        # ================= MLP (bf16, windowed interleave) ===================
        with tc.tile_pool(name="psum2", bufs=2, space="PSUM") as psum2:
            opacc = [psum2.tile([P, TQ], f32, tag="op", bufs=NCH,
                                name=f"op{ot}") for ot in range(NCH)]
            NG = 4
            GW = NFT // NG  # 6 hidden chunks per window

            def mlp_out(ot):
                ot_s = st([P, TQ], f32, "outt", 2, f"ot{ot}")
                nc.vector.tensor_scalar(ot_s, opacc[ot], 1.0,
                                        b2_s[:, ot:ot + 1], OP.mult, OP.add)
                nc.vector.tensor_tensor(ot_s, ot_s, x2t[ot], OP.add)
                nc.sync.dma_start(outT[ot * P:(ot + 1) * P, :], ot_s)

            for gw_i in range(NG):
                w2g = []
                for ot in range(NCH):
                    w2t = st([P, GW, P], bf16, "w2g", 7, f"w2g{gw_i}_{ot}")
                    nc.sync.dma_start(
                        w2t, w2B[ot, :, gw_i * GW:(gw_i + 1) * GW, :])
                    w2g.append(w2t)
                gts = []
                for mi in range(GW):
                    mt = gw_i * GW + mi
                    w1 = st([P, NCH, P], bf16, "w15", 8, f"w1_{mt}")
                    nc.gpsimd.dma_start(w1, w1B[mt])
                    mp = psum2.tile([P, TQ], f32, tag="mm", bufs=2,
                                    name=f"mp{mt}")
                    for kc in range(NCH):
                        nc.tensor.matmul(mp, w1[:, kc, :], u2[kc],
                                         start=(kc == 0),
                                         stop=(kc == NCH - 1))
                    gs = st([P, TQ], bf16, "et", 16, f"gt{mt}")
                    nc.scalar.activation(gs, mp, AF.Gelu,
                                         bias=b1_s[:, mt:mt + 1], scale=1.0)
                    gts.append(gs)
                for ot in range(NCH):
                    for mi in range(GW):
                        nc.tensor.matmul(
                            opacc[ot], w2g[ot][:, mi, :], gts[mi],
                            start=(gw_i == 0 and mi == 0),
                            stop=(gw_i == NG - 1 and mi == GW - 1))
                    if gw_i == NG - 1:
                        # final window: drain each output chunk as soon as
                        # its accumulation closes
                        mlp_out(ot)


def _get_nc():
    if "nc" not in _CACHE:
        _CACHE["nc"] = _build_nc()
    return _CACHE["nc"]


def _host_prep(inputs):
    import ml_dtypes
    bf = ml_dtypes.bfloat16
    f8 = ml_dtypes.float8_e4m3

    x = np.asarray(inputs["x"], np.float32)
    cond_len = int(np.asarray(inputs["cond_len"]))
    pm = np.asarray(inputs["padding_mask"])
    g1 = np.asarray(inputs["g1"], np.float32)
    bln1 = np.asarray(inputs["bln1"], np.float32)
    g2 = np.asarray(inputs["g2"], np.float32)
    bln2 = np.asarray(inputs["bln2"], np.float32)
    Wq = np.asarray(inputs["Wq"], np.float32)
    Wk = np.asarray(inputs["Wk"], np.float32)
    Wv = np.asarray(inputs["Wv"], np.float32)
    Wp = np.asarray(inputs["Wp"], np.float32)
    W1 = np.asarray(inputs["W1"], np.float32)
    W2 = np.asarray(inputs["W2"], np.float32)
    bq = np.asarray(inputs["bq"], np.float32)
    bk = np.asarray(inputs["bk"], np.float32)
    bv = np.asarray(inputs["bv"], np.float32)
    bp = np.asarray(inputs["bp"], np.float32)
    b1 = np.asarray(inputs["b1"], np.float32)
    b2 = np.asarray(inputs["b2"], np.float32)

    Wq_ = Wq * g1[None, :]
    Wk_ = Wk * g1[None, :]
    Wv_ = Wv * g1[None, :]
    bq_ = Wq @ bln1 + bq
    bk_ = Wk @ bln1 + bk
    bv_ = Wv @ bln1 + bv
    bp_ = bp + Wp @ bv_
    W1_ = W1 * g2[None, :]
    b1_ = W1 @ bln2 + b1

    def blk8(WT):
        # WT [K, M] -> [M/128, 128(kp), K/128, 128(m)], fp8 with x64 scale
        Kd, Md = WT.shape
        return np.ascontiguousarray(
            (WT * WSC).reshape(Kd // P, P, Md // P, P).transpose(2, 1, 0, 3)
        ).astype(f8)

    def blk16(WT):
        Kd, Md = WT.shape
        return np.ascontiguousarray(
            WT.reshape(Kd // P, P, Md // P, P).transpose(2, 1, 0, 3)
        ).astype(bf)

    def bre(b):
        return b.reshape(-1, P).T.astype(np.float32)

    wvP = np.ascontiguousarray(
        (Wv_.T * WSC).reshape(NP, 2, P, C).transpose(0, 2, 1, 3)).astype(f8)

    # selpack: slot 0 = sel (rows 0-1), slots 1-4 = selq one-hots
    spk = np.zeros((65, 5, P), np.float32)
    spk[0, 0, 0:Dh] = 1.0
    spk[1, 0, Dh:2 * Dh] = 1.0
    spk[0, 1, :] = 1.0
    spk[64, 2, :] = 1.0
    spk[0, 3, :] = -1.0
    spk[64, 4, :] = -1.0
    spk = spk.astype(bf)

    n_b = T - pm.sum(axis=1)
    cols = np.arange(T)
    allowed = (cols[None, :] >= cond_len) | (cols[None, :] < np.asarray(n_b)[:, None])
    M = allowed.astype(np.float32)

    shared = dict(
        wqB=blk8(Wq_.T), wkB=blk8(Wk_.T), wvP=wvP,
        wpB=blk8(Wp.T), w1B=blk16(W1_.T), w2B=blk16(W2.T),
        spk=spk)

    in_maps = []
    perms = []
    for core in range(N_CORES):
        b = core // 4
        qi = core % 4
        own = np.arange(qi * TQ, (qi + 1) * TQ)
        rest = np.concatenate([np.arange(0, qi * TQ), np.arange((qi + 1) * TQ, T)])
        perm = np.concatenate([own, rest])
        perms.append((b, qi))
        xb = x[b]
        m = dict(shared)
        mperm = M[b][perm] * WDESC
        cpk = np.zeros((P, 64), np.float32)
        cpk[:, CP_MB:CP_MB + NKT] = mperm.reshape(NKT, P).T
        cpk[:, CP_BQ:CP_BQ + NCH] = bre(bq_)
        cpk[:, CP_BK:CP_BK + NCH] = bre(bk_)
        cpk[:, CP_B1:CP_B1 + NFT] = bre(b1_)
        cpk[:, CP_B2:CP_B2 + NCH] = bre(b2)
        m.update(
            xT=np.ascontiguousarray(xb[perm].T).astype(bf),
            xTown=np.ascontiguousarray(xb[own].T + bp_[:, None]).astype(np.float32),
            cpk=np.ascontiguousarray(cpk))
        in_maps.append(m)
    return in_maps, perms


def kernel(**inputs):
    from concourse.bass_utils import run_bass_kernel_spmd

    nc = _get_nc()
    in_maps, perms = _host_prep(inputs)
    res = run_bass_kernel_spmd(nc, in_maps, list(range(N_CORES)),
                               **_CACHE.get("run_kwargs", {}))
    _CACHE["last_results"] = res
    x = np.asarray(inputs["x"])
    out = np.zeros((B, T, C), np.float32)
    for core in range(N_CORES):
        b, qi = perms[core]
        out[b, qi * TQ:(qi + 1) * TQ, :] = res.results[core]["outT"].T
    return out.astype(x.dtype)
